# revision 102
# baseline (speedup 1.0000x reference)
"""DCN-FPN Trainium2 kernel (nn_DCNFPN), v2.

Sharding: 8 cores = 4 images x 2 row-halves. Each core computes rows
[g0, g0+23] of every 40-row intermediate (g0 = 0 top / 16 bottom), with
shrinking-validity redundancy so no cross-core communication is needed;
host keeps rows 0..19 (top) / 20..39 (bottom) of the output.

Key structure (vs v1): the DRAM feature table packs the full 2x2
bilinear patch per entry -- entry (yy, xx) of an (H+1)x(W+1) grid holds
[f[yy-1,xx-1], f[yy-1,xx], f[yy,xx-1], f[yy,xx]] over 256 channels
(zero-filled out of bounds), 2 KB each.  One dma_gather per half-tap
(z-block, 512 idx) fetches all four corners; OOB x/y handling collapses
into table zeros plus one per-axis clamp-indicator folded into the
mask.  The four slot weights (A0,A1)x(xs0,xs1) are broadcast to 128
partitions through the PE (one-hot selector matmuls from the wall tile)
and copied PSUM->SBUF bf16 by the Activation engine -- no DRAM round
trip.  Corner combine per z-unit: 1 in-place TT mul (hl via 0-stride
view) + 1 q add on DVE (bf16, 2x), then 8 PSUM-accumulating matmuls
(the x-pair sum is folded into the matmul accumulation).  All 32 gather
dispatches are emitted up-front so Pool paces them purely by buffer
WAR; f master is bf16-only (h1 add on Pool); om->pos0/m32 shuffles are
PE permutation matmuls; gather indices replicate via a DRAM staging
tile + 8 parallel fills; fh is pre-accumulated into the residual-conv
PSUM by an identity matmul.

Per call: offset conv (36 mm, ih-outer) -> om copy/sigmoid -> perm mms
-> small math ([64,480]: trunc-floor/frac/clamp/valid; walls+idx on
[32,480]) -> idx i16 wrap via DRAM -> 16-tap/32-unit pipeline ->
f += relu(dc).  Final: residual conv (+fh in PSUM), store [256,960].

Sample enumeration per tap: gather column i = 512*z + 16*cc + p
(z = rc//480, p = rc%16, cc = (rc%480)//16); columns 480:512 of each
512-block are pad (idx 0, ignored).
"""
import sys
sys.path.insert(0, "/opt/trn_rl_repo")

from contextlib import ExitStack
import numpy as np
import ml_dtypes

import bass_rust
import concourse.bass as bass
import concourse.bacc as bacc
import concourse.mybir as mybir
import concourse.tile as tile

F32 = mybir.dt.float32
BF16 = mybir.dt.bfloat16
I16 = mybir.dt.int16
I32 = mybir.dt.int32
A = mybir.AluOpType
AF = mybir.ActivationFunctionType

B, C, HOUT = 4, 256, 40
CONFIGS = [(4, 2, 1, 1), (4, 4, 3, 3)]   # (k, stride, pad, dil)
HIN = [80, 160]                          # per level l=0 (f1), l=1 (f0)
TW = [HIN[0] + 1, HIN[1] + 1]            # packed-table grid width per level
ROWS = 24                                # out rows per core per call
RC = ROWS * HOUT                         # 960
NT = 16                                  # taps
CALLS = [0, 1, 0, 1]
FW = 42                                  # padded f width
FR = 26                                  # f window rows
FSZ = FR * FW                            # 1092


def vp(ap, dims, doff=0):
    v = ap.copy()
    v.ap = bass_rust.VecI64Pair(dims)
    if doff:
        v.offset = v.offset + doff
    return v


def build_program():
    nc = bacc.Bacc("TRN2", target_bir_lowering=False, debug=False)

    dt = {}

    def din(name, shape, dtype=F32):
        dt[name] = nc.dram_tensor(name, shape, dtype, kind="ExternalInput").ap()

    din("fp0", [TW[1] * TW[1], 1024], BF16)   # level 1 packed table (f0)
    din("fp1", [TW[0] * TW[0], 1024], BF16)   # level 0 packed table (f1)
    din("finit", [C, FSZ], BF16)
    din("fh", [128, 2 * RC], BF16)
    din("byx", [64, 2 * 480], F32)
    din("hi0", [64, 2], F32)
    din("sel", [32, 32 * 128], BF16)
    din("pperm", [32, 4 * 32], F32)
    din("mperm", [16, 2 * 32], F32)
    din("ident", [128, 128], BF16)
    din("com_w", [128, 2 * 9 * 2 * 48], BF16)
    din("com_b", [48, 2], F32)
    din("dcn_w", [2, 128, NT * 2 * 2 * 128], BF16)
    din("dcn_b", [128, 4], F32)
    din("res_w", [128, 9 * 2 * 2 * 128], BF16)
    din("res_b", [128, 2], F32)
    out_d = nc.dram_tensor("out", [C, RC], F32, kind="ExternalOutput").ap()

    with tile.TileContext(nc) as tc, ExitStack() as ctx:
        build_body(nc, tc, ctx, dt, out_d)
    nc.compile()
    return nc


def build_body(nc, tc, ctx, dt, out_d):
    cst = ctx.enter_context(tc.tile_pool(name="cst", bufs=1))
    s64p = ctx.enter_context(tc.tile_pool(name="s64p", bufs=4))
    s32p = ctx.enter_context(tc.tile_pool(name="s32p", bufs=4))
    smi = ctx.enter_context(tc.tile_pool(name="smi", bufs=1))
    omp = ctx.enter_context(tc.tile_pool(name="omp", bufs=1))
    wgt = ctx.enter_context(tc.tile_pool(name="wgt", bufs=1))
    walp = ctx.enter_context(tc.tile_pool(name="walp", bufs=1))
    wbp = ctx.enter_context(tc.tile_pool(name="wbp", bufs=3))
    gat = ctx.enter_context(tc.tile_pool(name="gat", bufs=6))
    qp = ctx.enter_context(tc.tile_pool(name="qp", bufs=4))
    fup = ctx.enter_context(tc.tile_pool(name="fup", bufs=2))
    pso = ctx.enter_context(tc.tile_pool(name="pso", bufs=2, space="PSUM"))
    psd = ctx.enter_context(tc.tile_pool(name="psd", bufs=1, space="PSUM"))
    drp = ctx.enter_context(tc.tile_pool(name="drp", bufs=2, space="DRAM"))

    # ---- persistent loads (critical first; spread across SP/Act queues) --
    com_t = cst.tile([128, 2 * 9 * 2 * 48], BF16, tag="com")
    nc.sync.dma_start(com_t[:], dt["com_w"])
    com_v = com_t[:].rearrange("p (l t i o) -> p l t i o", l=2, t=9, i=2, o=48)

    fsh = []
    for h in range(2):
        fs = cst.tile([128, FSZ], BF16, tag=f"fsh{h}")
        nc.sync.dma_start(fs[:], dt["finit"][128 * h:128 * (h + 1), :])
        fsh.append(fs)

    byx_t = cst.tile([64, 2 * 480], F32, tag="byx")
    nc.scalar.dma_start(byx_t[:], dt["byx"])
    hi0_t = cst.tile([64, 2], F32, tag="hi0")
    nc.scalar.dma_start(hi0_t[:], dt["hi0"])
    comb_t = cst.tile([48, 2], F32, tag="comb")
    nc.scalar.dma_start(comb_t[:], dt["com_b"])
    sel_t = cst.tile([32, 32 * 128], BF16, tag="sel")
    nc.scalar.dma_start(sel_t[:], dt["sel"])
    sel_v = sel_t[:].rearrange("p (r o) -> p r o", r=32)
    pperm_t = cst.tile([32, 4 * 32], F32, tag="pperm")
    nc.scalar.dma_start(pperm_t[:], dt["pperm"])
    pperm_v = pperm_t[:].rearrange("p (v o) -> p v o", v=4)
    mperm_t = cst.tile([16, 2 * 32], F32, tag="mperm")
    nc.scalar.dma_start(mperm_t[:], dt["mperm"])
    mperm_v = mperm_t[:].rearrange("p (v o) -> p v o", v=2)
    ident_t = cst.tile([128, 128], BF16, tag="ident")
    nc.scalar.dma_start(ident_t[:], dt["ident"])
    dcnb_t = cst.tile([128, 4], F32, tag="dcnb")
    nc.scalar.dma_start(dcnb_t[:], dt["dcn_b"])
    resb_t = cst.tile([128, 2], F32, tag="resb")
    nc.scalar.dma_start(resb_t[:], dt["res_b"])
    fh_t = cst.tile([128, 2 * RC], BF16, tag="fh")
    nc.scalar.dma_start(fh_t[:], dt["fh"])

    # per-level DCN weights, loaded once
    dcn_ts = []
    for lvl in range(2):
        t_ = cst.tile([128, NT * 2 * 2 * 128], BF16, tag=f"dcn{lvl}")
        nc.scalar.dma_start(t_[:], dt["dcn_w"][lvl])
        dcn_ts.append(t_[:].rearrange("p (k i o q) -> p k i o q",
                                      k=NT, i=2, o=2, q=128))

    fp_ap = {0: dt["fp1"], 1: dt["fp0"]}

    # DRAM staging tile for wrapped gather indices; zero it once so the
    # per-(t,z) pad lanes (cc 30:32) read as index 0 in every call.
    repD = drp.tile([16, NT * 64], I16, tag="repD")
    zs16 = smi.tile([16, NT * 64], I16, tag="zs16")
    nc.vector.memset(zs16[:], 0)
    nc.sync.dma_start(repD[:], zs16[:])

    # ---- per-call ---------------------------------------------------------
    for ci, lvl in enumerate(CALLS):
        Win = HIN[lvl]
        Wt = TW[lvl]
        dcn_v = dcn_ts[lvl]

        # offset conv: om_ps rows 0:48, (z,512)-chunked, 480 used
        om_ps = pso.tile([128, 1024], F32, tag="ps", name=f"omps_{ci}")
        conv3x3(nc, fsh, lambda ti, ih: com_v[:, lvl, ti, ih], om_ps, rows=48)

        # mask activation (com_b offset-bias is folded into byx host-side)
        m16 = omp.tile([16, RC], F32, tag="m16")
        omv1 = om_ps[32:48, :].rearrange("p (z c) -> p z c", z=2)[:, :, 0:480]
        nc.scalar.activation(m16[:], omv1, AF.Sigmoid,
                             bias=comb_t[32:48, lvl:lvl + 1])

        # stage offsets PSUM->SBUF, then shuffle into [64,480]
        # (p = yx*32 + rcb*16 + t) / [32,480] via PE permutation matmuls
        om01 = omp.tile([32, RC], F32, tag="om01")
        omv0 = om_ps[0:32, :].rearrange("p (z c) -> p z c", z=2)[:, :, 0:480]
        nc.scalar.activation(om01[:], omv0, AF.Copy)
        pos0ps = pso.tile([128, 1024], F32, tag="ps", name=f"pos0ps_{ci}")
        for yx in range(2):
            for rcb in range(2):
                nc.tensor.matmul(
                    pos0ps[yx * 32:(yx + 1) * 32, 0:480],
                    pperm_v[:, yx * 2 + rcb, :],
                    om01[0:32, rcb * 480:(rcb + 1) * 480],
                    start=(rcb == 0), stop=(rcb == 1))
        pos0 = pos0ps[0:64, 0:480]
        m32ps = pso.tile([128, 1024], F32, tag="ps", name=f"m32ps_{ci}")
        for rcb in range(2):
            nc.tensor.matmul(m32ps[0:32, 0:480], mperm_v[:, rcb, :],
                             m16[:, rcb * 480:(rcb + 1) * 480],
                             start=(rcb == 0), stop=(rcb == 1))
        m32 = m32ps[0:32, 0:480]

        # ---- small math ----
        cnt = [0]

        def t64():
            cnt[0] += 1
            return s64p.tile([64, 480], F32, tag="s64", name=f"t64_{ci}_{cnt[0]}")

        def t32():
            cnt[0] += 1
            return s32p.tile([32, 480], F32, tag="s32", name=f"t32_{ci}_{cnt[0]}")

        def t64i():
            cnt[0] += 1
            return s64p.tile([64, 480], I32, tag="s64i", bufs=1,
                             name=f"t64i_{ci}_{cnt[0]}")

        # positions carry a +1+1024 shift (baked into byx): +1 for the grid,
        # +1024 so floor-via-mod sees positive operands on hardware.
        # --- idx-critical path first (high priority: gathers wait on it) ---
        hp = tc.high_priority()
        hp.__enter__()
        sh = t64()
        nc.vector.tensor_tensor(sh[:], pos0,
                                byx_t[:, lvl * 480:(lvl + 1) * 480], A.add)
        i32t = t64i()
        nc.vector.tensor_copy(i32t[:], sh[:])
        ff = t64()
        nc.vector.tensor_copy(ff[:], i32t[:])
        gt = t64()
        nc.vector.tensor_tensor(gt[:], ff[:], sh[:], A.is_gt)
        fls = t64()
        nc.vector.tensor_tensor(fls[:], ff[:], gt[:], A.subtract)
        c0 = t64()
        nc.vector.tensor_scalar(c0[:], fls[:], 1024.0, hi0_t[:, lvl:lvl + 1],
                                A.max, A.min)
        # gather idx = (c0y-1024)*Wt + c0x-1024  (psx pre-subtracts the shift)
        psx = t32()
        nc.vector.tensor_scalar(psx[:], c0[32:64, :],
                                -1024.0 * (Wt + 1.0), None, A.add)
        gyt = t32()
        nc.vector.scalar_tensor_tensor(gyt[:], c0[0:32, :], float(Wt),
                                       psx[:], A.mult, A.add)
        i16t = smi.tile([32, 480], I16, tag="i16")
        nc.vector.tensor_copy(i16t[:], gyt[:])
        dflat = drp.tile([32, 480], I16, tag="dfl")
        nc.sync.dma_start(dflat[:], i16t[:])

        # idx wrap via DRAM: repD[p', t*64+z*32+cc] = dflat[(z*16+t)*480
        # + cc*16 + p'] (DRAM->DRAM strided, chunked by tap-half x z on two
        # queues), then broadcast DMAs fill the 8 replica row-groups.
        # repD pad lanes (cc 30:32) are zeroed once at kernel start.
        dfv = dflat[:].rearrange("p c -> (p c)")
        rdv = repD[:].rearrange("p (t z cc) -> p t z cc", t=NT, z=2, cc=32)
        HT = NT // 2
        for th, eng in ((0, nc.sync), (1, nc.scalar)):
            for z in range(2):
                wrap = smi.tile([16, HT * 30], I16, tag=f"wrap{th}{z}",
                                name=f"wrap_{ci}_{th}_{z}")
                base = (z * 16 + th * HT) * 480
                src = dfv[base:base + HT * 480]
                src = src.rearrange("(tc p) -> p tc", p=16)
                eng.dma_start(wrap[:], src)
                wv_ = wrap[:].rearrange("p (t cc) -> p t cc", t=HT)
                eng.dma_start(rdv[0:16, th * HT:(th + 1) * HT, z, 0:30], wv_)
        rep = smi.tile([128, NT * 64], I16, tag="rep")
        for grp in range(8):
            eng = nc.sync if grp % 2 == 0 else nc.scalar
            eng.dma_start(rep[grp * 16:(grp + 1) * 16, :], repD[:])
        hp.__exit__(None, None, None)

        # --- weight path (overlaps the idx DMA chain) ---
        frac = t64()
        nc.vector.tensor_tensor(frac[:], sh[:], fls[:], A.subtract)
        V = t64()
        nc.vector.tensor_tensor(V[:], c0[:], fls[:], A.is_equal)
        u = t64()
        nc.vector.tensor_scalar(u[:], frac[:], -1.0, 1.0, A.mult, A.add)

        # mask' = m * Vy * Vx  (x rows copied down to base partition 0;
        # weight-path copies on Act, off the DVE critical path)
        vx32 = t32()
        nc.scalar.copy(vx32[:], V[32:64, :])
        mv = t32()
        nc.vector.tensor_tensor(mv[:], m32, V[0:32, :], A.mult)
        mm_ = t32()
        nc.vector.tensor_tensor(mm_[:], mv[:], vx32[:], A.mult)
        A0 = t32()
        nc.vector.tensor_tensor(A0[:], u[0:32, :], mm_[:], A.mult)
        A1 = t32()
        nc.vector.tensor_tensor(A1[:], frac[0:32, :], mm_[:], A.mult)
        xs0 = t32()
        nc.scalar.copy(xs0[:], u[32:64, :])
        xs1 = t32()
        nc.scalar.copy(xs1[:], frac[32:64, :])

        # wall [32, (cy, px, 480)] bf16
        wall = walp.tile([32, 4 * 480], BF16, tag="wall")
        nc.vector.tensor_tensor(wall[:, 0 * 480:1 * 480], A0[:], xs0[:], A.mult)
        nc.vector.tensor_tensor(wall[:, 1 * 480:2 * 480], A0[:], xs1[:], A.mult)
        nc.vector.tensor_tensor(wall[:, 2 * 480:3 * 480], A1[:], xs0[:], A.mult)
        nc.vector.tensor_tensor(wall[:, 3 * 480:4 * 480], A1[:], xs1[:], A.mult)

        # dc accumulator [2][128, 1024] ((z,512)-chunked, 480 used)
        dcs = [psd.tile([128, 1024], F32, tag=f"dc{oh}", name=f"dc_{ci}_{oh}")
               for oh in range(2)]

        fpv = fp_ap[lvl]

        def emit_bcast(t):
            # PE broadcast via one-hot selector: bc[o,c] = wall[t+16z, c]
            wallb = wbp.tile([128, 4 * 960], BF16, tag="wallb",
                             name=f"wallb_{ci}_{t}")
            for j in range(4):
                bc = pso.tile([128, 1024], F32, tag="ps", name=f"bc_{ci}_{t}_{j}")
                for z in range(2):
                    nc.tensor.matmul(bc[:, z * 512:z * 512 + 480],
                                     sel_v[:, t + 16 * z, :],
                                     wall[0:32, j * 480:(j + 1) * 480],
                                     start=True, stop=True)
                bcv = bc[:].rearrange("p (z c) -> p z c", z=2)[:, :, 0:480]
                wbv = wallb[:, j * 960:(j + 1) * 960].rearrange(
                    "p (z c) -> p z c", z=2)
                nc.scalar.activation(wbv, bcv, AF.Copy)
            return wallb

        def emit_gather(t, z):
            # gather: one 2KB element per sample = full 2x2 patch; half-tap
            g = gat.tile([128, 8 * 512], BF16, tag="g", name=f"g_{ci}_{t}_{z}")
            gv = g[:].rearrange("p (j i) -> p j i", j=8)
            nc.gpsimd.dma_gather(gv, fpv,
                                 rep[:, t * 64 + z * 32:t * 64 + z * 32 + 32],
                                 512, 512, 1024, transpose=True,
                                 single_packet=False)
            return g

        # all gather dispatches up-front: Pool's in-order queue paces them
        # purely by gat-buffer WAR, never behind a compute op
        gs = {(t, z): emit_gather(t, z) for t in range(NT) for z in range(2)}
        wallbs = {0: emit_bcast(0)}
        for t in range(NT):
            if t + 1 < NT:
                wallbs[t + 1] = emit_bcast(t + 1)
            wallb = wallbs.pop(t)
            for z in range(2):
                g = gs.pop((t, z))
                gb = g[:]

                # in-place mul: p = g * wall  (one op, both corners)
                pv = vp(gb, [[4096, 128], [2048, 2], [1024, 2], [512, 2],
                             [1, 480]])
                wv = vp(wallb[:], [[3840, 128], [1920, 2], [960, 2], [0, 2],
                                   [1, 480]], doff=z * 480)
                nc.vector.tensor_tensor(pv, pv, wv, A.mult)

                # q = p[cy0] + p[cy1]   [128, (px, hl, 480)]
                # z0 on DVE, z1 on Pool
                q = qp.tile([128, 2 * 960], BF16, tag="q",
                            name=f"q_{ci}_{t}_{z}")
                qv = vp(q[:], [[1920, 128], [960, 2], [480, 2], [1, 480]])
                pa = vp(gb, [[4096, 128], [1024, 2], [512, 2], [1, 480]])
                pb = vp(gb, [[4096, 128], [1024, 2], [512, 2], [1, 480]],
                        doff=2048)
                nc.vector.tensor_tensor(qv, pa, pb, A.add)

                # s-sum folded into the matmuls: feed both px halves of q
                qview = q[:].rearrange("p (x h c) -> p x h c", x=2, h=2)
                for oh in range(2):
                    for ih in range(2):
                        for px in range(2):
                            nc.tensor.matmul(
                                dcs[oh][:, z * 512:z * 512 + 480],
                                dcn_v[:, t, ih, oh],
                                qview[:, px, ih, :],
                                start=(t == 0 and ih == 0 and px == 0),
                                stop=(t == NT - 1 and ih == 1 and px == 1))

        # f update: f += relu(dc + b)   (bf16 master; h1 add on Pool so the
        # two halves update in parallel and the conv starts sooner)
        for h in range(2):
            rel = fup.tile([128, RC], BF16, tag="rel", name=f"rel_{ci}_{h}")
            dcv = dcs[h][:].rearrange("p (z c) -> p z c", z=2)[:, :, 0:480]
            nc.scalar.activation(rel[:], dcv, AF.Relu,
                                 bias=dcnb_t[:, 2 * lvl + h:2 * lvl + h + 1])
            fsv = fsh[h][:].rearrange("p (r c) -> p r c", c=FW)[:, 1:25, 1:41]
            rv = rel[:].rearrange("p (r c) -> p r c", c=HOUT)
            (nc.vector if h == 0 else nc.gpsimd).tensor_tensor(
                fsv, fsv, rv, A.add)

    # ---- residual conv + fh ----------------------------------------------
    # fh is pre-accumulated into the PSUM via an identity matmul (start),
    # then the conv taps accumulate on top; output = act(psum + bias).
    res_t = wgt.tile([128, 9 * 2 * 2 * 128], BF16, tag="res")
    nc.sync.dma_start(res_t[:], dt["res_w"])
    res_v = res_t[:].rearrange("p (t i o q) -> p t i o q", t=9, i=2, o=2)
    for oh in range(2):
        rps = psd.tile([128, 1024], F32, tag=f"dc{oh}", name=f"rps_{oh}")
        fhv = fh_t[:].rearrange("p (o z c) -> p o z c", o=2, z=2)
        for z in range(2):
            nc.tensor.matmul(rps[:, z * 512:z * 512 + 480], ident_t[:],
                             fhv[:, oh, z, :], start=True, stop=False)
        conv3x3(nc, fsh, lambda ti, ih, oh=oh: res_v[:, ti, ih, oh], rps,
                accum=True)
        ot = fup.tile([128, RC], F32, tag="ot")
        rpv = rps[:].rearrange("p (z c) -> p z c", z=2)[:, :, 0:480]
        nc.scalar.activation(ot[:], rpv, AF.Identity, bias=resb_t[:, oh:oh + 1])
        nc.sync.dma_start(out_d[128 * oh:128 * (oh + 1), :], ot[:])


def conv3x3(nc, fsh, w_fn, out_ps, rows=128, accum=False):
    """3x3 stride-1 conv over the padded f window; out [rows, (z,512|480)].

    ih-outer so the ih=0 matmuls can start before fsh[1] is updated."""
    taps = [(a, b) for a in (-1, 0, 1) for b in (-1, 0, 1)]
    for ih in range(2):
        rhs = fsh[ih][:].rearrange("p (r c) -> p r c", c=FW)
        for ti, (dy, dx) in enumerate(taps):
            for nh in range(2):
                nc.tensor.matmul(
                    out_ps[0:rows, nh * 512:nh * 512 + 480],
                    w_fn(ti, ih),
                    rhs[:, 1 + dy + nh * 12:1 + dy + nh * 12 + 12,
                        1 + dx:1 + dx + 40],
                    start=(not accum and ih == 0 and ti == 0),
                    stop=(ih == 1 and ti == 8))


# ===========================================================================
# host side
# ===========================================================================

def packed_table(f):
    """[(H+1)*(W+1), 1024] bf16: entry (yy,xx) = 2x2 patch at (yy-1, xx-1)."""
    Cc, H, W = f.shape
    fpad = np.zeros((Cc, H + 2, W + 2), np.float32)
    fpad[:, 1:H + 1, 1:W + 1] = f
    parts = [fpad[:, dy:dy + H + 1, dx:dx + W + 1]
             for dy, dx in ((0, 0), (0, 1), (1, 0), (1, 1))]
    t = np.stack(parts, axis=0)            # [4, C, H+1, W+1]
    t = t.transpose(2, 3, 0, 1)            # [H+1, W+1, 4, C]
    return np.ascontiguousarray(
        t.reshape((H + 1) * (W + 1), 4 * Cc)).astype(ml_dtypes.bfloat16)


def prep_core_inputs(inputs, b, half):
    """Per-core input map for image b, row-half `half` (0=top)."""
    g0 = 0 if half == 0 else 16
    f0 = np.asarray(inputs["f0"][b], np.float32)
    f1 = np.asarray(inputs["f1"][b], np.float32)
    f2 = np.asarray(inputs["f2"][b], np.float32)

    finit = np.zeros((C, FR, FW), np.float32)
    for r in range(FR):
        gr = g0 - 1 + r
        if 0 <= gr < HOUT:
            finit[:, r, 1:41] = f2[:, gr, :]

    # fh as [128, (oh, rc)]
    fh0 = f2[:, g0:g0 + ROWS, :].reshape(C, RC)
    fh = np.concatenate([fh0[:128], fh0[128:]], axis=1)

    perm = list(range(0, 32, 2)) + list(range(1, 32, 2)) + list(range(32, 48))

    # base positions in +1-shifted grid coords; offset-conv bias folded in
    byx = np.zeros((2, 64, 480), np.float32)
    hi0 = np.zeros((2, 64, 1), np.float32)
    for lvl in range(2):
        k_, st_, pad_, dil_ = CONFIGS[lvl]
        Hin = HIN[lvl]
        cbp = np.asarray(inputs[f"com_b{lvl}"], np.float32)[perm]
        rc = np.arange(480)
        for rcb in range(2):
            rr = (rcb * 480 + rc) // HOUT
            cc = (rcb * 480 + rc) % HOUT
            for t in range(NT):
                byx[lvl, rcb * 16 + t] = (st_ * (g0 + rr) - pad_
                                          + (t // k_) * dil_ + 1025 + cbp[t])
                byx[lvl, 32 + rcb * 16 + t] = (st_ * cc - pad_ + (t % k_) * dil_
                                               + 1025 + cbp[16 + t])
        hi0[lvl, 0:32] = 1024 + Hin   # clamp hi in shifted coords
        hi0[lvl, 32:64] = 1024 + Hin
    byx = byx.transpose(1, 0, 2).reshape(64, 2 * 480)
    hi0 = hi0.transpose(1, 0, 2).reshape(64, 2)
    com_w = np.zeros((2, 9, 2, 128, 48), np.float32)
    com_b = np.zeros((2, 48, 1), np.float32)
    dcn_w = np.zeros((2, NT, 2, 2, 128, 128), np.float32)
    dcn_b = np.zeros((2, 2, 128, 1), np.float32)
    for lvl in range(2):
        cw = np.asarray(inputs[f"com_w{lvl}"], np.float32)[perm]
        cb = np.asarray(inputs[f"com_b{lvl}"], np.float32)[perm]
        for ty in range(3):
            for tx in range(3):
                for ih in range(2):
                    com_w[lvl, ty * 3 + tx, ih] = \
                        cw[:, ih * 128:(ih + 1) * 128, ty, tx].T
        com_b[lvl, :, 0] = cb
        dw = np.asarray(inputs[f"dcn_w{lvl}"], np.float32)
        for k in range(NT):
            for ih in range(2):
                for oh in range(2):
                    dcn_w[lvl, k, ih, oh] = dw[oh * 128:(oh + 1) * 128,
                                               ih * 128:(ih + 1) * 128,
                                               k // 4, k % 4].T
        db = np.asarray(inputs[f"dcn_b{lvl}"], np.float32)
        dcn_b[lvl, 0, :, 0] = db[:128]
        dcn_b[lvl, 1, :, 0] = db[128:]
    rw = np.asarray(inputs["res_w"], np.float32)
    res_w = np.zeros((9, 2, 2, 128, 128), np.float32)
    for ty in range(3):
        for tx in range(3):
            for ih in range(2):
                for oh in range(2):
                    res_w[ty * 3 + tx, ih, oh] = rw[oh * 128:(oh + 1) * 128,
                                                    ih * 128:(ih + 1) * 128,
                                                    ty, tx].T
    rb = np.asarray(inputs["res_b"], np.float32)
    res_b = np.stack([rb[:128], rb[128:]], axis=1)  # [128, 2]

    # PE permutation matrices for the om -> pos0 / m16 -> m32 shuffles
    pperm = np.zeros((32, 4, 32), np.float32)
    for yx in range(2):
        for rcb in range(2):
            for t in range(16):
                pperm[yx * 16 + t, yx * 2 + rcb, rcb * 16 + t] = 1.0
    mperm = np.zeros((16, 2, 32), np.float32)
    for rcb in range(2):
        for t in range(16):
            mperm[t, rcb, rcb * 16 + t] = 1.0

    com_w = com_w.transpose(3, 0, 1, 2, 4).reshape(128, -1)
    com_b = com_b.transpose(1, 0, 2).reshape(48, 2)
    dcn_w = dcn_w.transpose(0, 4, 1, 2, 3, 5).reshape(2, 128, -1)
    dcn_b = dcn_b.transpose(2, 0, 1, 3).reshape(128, 4)
    res_w = res_w.transpose(3, 0, 1, 2, 4).reshape(128, -1)

    return {
        "fp0": packed_table(f0),
        "fp1": packed_table(f1),
        "finit": finit.reshape(C, FSZ).astype(ml_dtypes.bfloat16),
        "fh": fh.astype(ml_dtypes.bfloat16),
        "byx": byx,
        "hi0": hi0,
        "sel": np.ascontiguousarray(
            np.tile(np.eye(32, dtype=np.float32)[:, :, None],
                    (1, 1, 128)).reshape(32, 32 * 128)
        ).astype(ml_dtypes.bfloat16),
        "pperm": pperm.reshape(32, 4 * 32),
        "mperm": mperm.reshape(16, 2 * 32),
        "ident": np.eye(128, dtype=np.float32).astype(ml_dtypes.bfloat16),
        "com_w": com_w.astype(ml_dtypes.bfloat16),
        "com_b": np.ascontiguousarray(com_b),
        "dcn_w": np.ascontiguousarray(dcn_w).astype(ml_dtypes.bfloat16),
        "dcn_b": np.ascontiguousarray(dcn_b),
        "res_w": np.ascontiguousarray(res_w).astype(ml_dtypes.bfloat16),
        "res_b": np.ascontiguousarray(res_b).astype(np.float32),
    }


def assemble_output(results):
    out = np.zeros((B, C, HOUT, HOUT), np.float32)
    for b in range(B):
        top = np.asarray(results[2 * b]["out"]).reshape(C, ROWS, HOUT)
        bot = np.asarray(results[2 * b + 1]["out"]).reshape(C, ROWS, HOUT)
        out[b, :, 0:20, :] = top[:, 0:20, :]
        out[b, :, 20:40, :] = bot[:, 4:24, :]
    return out


_NC_CACHE = []


def kernel(**inputs):
    if not _NC_CACHE:
        _NC_CACHE.append(build_program())
    nc = _NC_CACHE[0]
    in_maps = [prep_core_inputs(inputs, b, half)
               for b in range(B) for half in range(2)]
    from concourse.bass_utils import run_bass_kernel_spmd
    r = run_bass_kernel_spmd(nc, in_maps, list(range(8)))
    return assemble_output(r.results)


# revision 108
# speedup vs baseline: 1.0241x; 1.0241x over previous
"""DCN-FPN Trainium2 kernel (nn_DCNFPN), v2.

Sharding: 8 cores = 4 images x 2 row-halves. Each core computes rows
[g0, g0+23] of every 40-row intermediate (g0 = 0 top / 16 bottom), with
shrinking-validity redundancy so no cross-core communication is needed;
host keeps rows 0..19 (top) / 20..39 (bottom) of the output.

Key structure (vs v1): the DRAM feature table packs the full 2x2
bilinear patch per entry -- entry (yy, xx) of an (H+1)x(W+1) grid holds
[f[yy-1,xx-1], f[yy-1,xx], f[yy,xx-1], f[yy,xx]] over 256 channels
(zero-filled out of bounds), 2 KB each.  One dma_gather per half-tap
(z-block, 512 idx) fetches all four corners; OOB x/y handling collapses
into table zeros plus one per-axis clamp-indicator folded into the
mask.  The four slot weights (A0,A1)x(xs0,xs1) are broadcast to 128
partitions through the PE (one-hot selector matmuls from the wall tile)
and copied PSUM->SBUF bf16 by the Activation engine -- no DRAM round
trip.  Corner combine per z-unit: 1 in-place TT mul (hl via 0-stride
view) + 1 q add on DVE (bf16, 2x), then 8 PSUM-accumulating matmuls
(the x-pair sum is folded into the matmul accumulation).  All 32 gather
dispatches are emitted up-front so Pool paces them purely by buffer
WAR; f master is bf16-only (h1 add on Pool); om->pos0/m32 shuffles are
PE permutation matmuls; gather indices replicate via a DRAM staging
tile + 8 parallel fills; fh is pre-accumulated into the residual-conv
PSUM by an identity matmul.

Per call: offset conv (36 mm, ih-outer) -> om copy/sigmoid -> perm mms
-> small math ([64,480]: trunc-floor/frac/clamp/valid; walls+idx on
[32,480]) -> idx i16 wrap via DRAM -> 16-tap/32-unit pipeline ->
f += relu(dc).  Final: residual conv (+fh in PSUM), store [256,960].

Sample enumeration per tap: gather column i = 512*z + 16*cc + p
(z = rc//480, p = rc%16, cc = (rc%480)//16); columns 480:512 of each
512-block are pad (idx 0, ignored).
"""
import sys
sys.path.insert(0, "/opt/trn_rl_repo")

from contextlib import ExitStack
import numpy as np
import ml_dtypes

import bass_rust
import concourse.bass as bass
import concourse.bacc as bacc
import concourse.mybir as mybir
import concourse.tile as tile

F32 = mybir.dt.float32
BF16 = mybir.dt.bfloat16
I16 = mybir.dt.int16
I32 = mybir.dt.int32
A = mybir.AluOpType
AF = mybir.ActivationFunctionType

B, C, HOUT = 4, 256, 40
CONFIGS = [(4, 2, 1, 1), (4, 4, 3, 3)]   # (k, stride, pad, dil)
HIN = [80, 160]                          # per level l=0 (f1), l=1 (f0)
TW = [HIN[0] + 1, HIN[1] + 1]            # packed-table grid width per level
ROWS = 24                                # out rows per core per call
RC = ROWS * HOUT                         # 960
NT = 16                                  # taps
CALLS = [0, 1, 0, 1]
FW = 42                                  # padded f width
FR = 26                                  # f window rows
FSZ = FR * FW                            # 1092


def vp(ap, dims, doff=0):
    v = ap.copy()
    v.ap = bass_rust.VecI64Pair(dims)
    if doff:
        v.offset = v.offset + doff
    return v


def build_program():
    nc = bacc.Bacc("TRN2", target_bir_lowering=False, debug=False)

    dt = {}

    def din(name, shape, dtype=F32):
        dt[name] = nc.dram_tensor(name, shape, dtype, kind="ExternalInput").ap()

    din("fp0", [TW[1] * TW[1], 1024], BF16)   # level 1 packed table (f0)
    din("fp1", [TW[0] * TW[0], 1024], BF16)   # level 0 packed table (f1)
    din("finit", [C, FSZ], BF16)
    din("fh", [128, 2 * RC], BF16)
    din("byx", [64, 2 * 480], F32)
    din("hi0", [64, 2], F32)
    din("sel", [32, 32 * 128], BF16)
    din("pperm", [32, 4 * 32], BF16)
    din("mperm", [16, 2 * 32], BF16)
    din("ident", [128, 128], BF16)
    din("com_w", [128, 2 * 9 * 2 * 48], BF16)
    din("com_b", [48, 2], F32)
    din("dcn_w", [2, 128, NT * 2 * 2 * 128], BF16)
    din("dcn_b", [128, 4], F32)
    din("res_w", [128, 9 * 2 * 2 * 128], BF16)
    din("res_b", [128, 2], F32)
    out_d = nc.dram_tensor("out", [C, RC], F32, kind="ExternalOutput").ap()

    with tile.TileContext(nc) as tc, ExitStack() as ctx:
        build_body(nc, tc, ctx, dt, out_d)
    nc.compile()
    return nc


def build_body(nc, tc, ctx, dt, out_d):
    cst = ctx.enter_context(tc.tile_pool(name="cst", bufs=1))
    s64p = ctx.enter_context(tc.tile_pool(name="s64p", bufs=4))
    s32p = ctx.enter_context(tc.tile_pool(name="s32p", bufs=4))
    smi = ctx.enter_context(tc.tile_pool(name="smi", bufs=1))
    omp = ctx.enter_context(tc.tile_pool(name="omp", bufs=1))
    wgt = ctx.enter_context(tc.tile_pool(name="wgt", bufs=1))
    walp = ctx.enter_context(tc.tile_pool(name="walp", bufs=1))
    wbp = ctx.enter_context(tc.tile_pool(name="wbp", bufs=3))
    gat = ctx.enter_context(tc.tile_pool(name="gat", bufs=6))
    qp = ctx.enter_context(tc.tile_pool(name="qp", bufs=4))
    fup = ctx.enter_context(tc.tile_pool(name="fup", bufs=2))
    pso = ctx.enter_context(tc.tile_pool(name="pso", bufs=2, space="PSUM"))
    psd = ctx.enter_context(tc.tile_pool(name="psd", bufs=1, space="PSUM"))
    drp = ctx.enter_context(tc.tile_pool(name="drp", bufs=2, space="DRAM"))

    # ---- persistent loads (critical first; spread across SP/Act queues) --
    com_t = cst.tile([128, 2 * 9 * 2 * 48], BF16, tag="com")
    nc.sync.dma_start(com_t[:], dt["com_w"])
    com_v = com_t[:].rearrange("p (l t i o) -> p l t i o", l=2, t=9, i=2, o=48)

    fsh = []
    for h in range(2):
        fs = cst.tile([128, FSZ], BF16, tag=f"fsh{h}")
        nc.sync.dma_start(fs[:], dt["finit"][128 * h:128 * (h + 1), :])
        fsh.append(fs)

    byx_t = cst.tile([64, 2 * 480], F32, tag="byx")
    nc.scalar.dma_start(byx_t[:], dt["byx"])
    hi0_t = cst.tile([64, 2], F32, tag="hi0")
    nc.scalar.dma_start(hi0_t[:], dt["hi0"])
    comb_t = cst.tile([48, 2], F32, tag="comb")
    nc.scalar.dma_start(comb_t[:], dt["com_b"])
    sel_t = cst.tile([32, 32 * 128], BF16, tag="sel")
    nc.scalar.dma_start(sel_t[:], dt["sel"])
    sel_v = sel_t[:].rearrange("p (r o) -> p r o", r=32)
    pperm_t = cst.tile([32, 4 * 32], BF16, tag="pperm")
    nc.scalar.dma_start(pperm_t[:], dt["pperm"])
    pperm_v = pperm_t[:].rearrange("p (v o) -> p v o", v=4)
    mperm_t = cst.tile([16, 2 * 32], BF16, tag="mperm")
    nc.scalar.dma_start(mperm_t[:], dt["mperm"])
    mperm_v = mperm_t[:].rearrange("p (v o) -> p v o", v=2)
    ident_t = cst.tile([128, 128], BF16, tag="ident")
    nc.scalar.dma_start(ident_t[:], dt["ident"])
    dcnb_t = cst.tile([128, 4], F32, tag="dcnb")
    nc.scalar.dma_start(dcnb_t[:], dt["dcn_b"])
    resb_t = cst.tile([128, 2], F32, tag="resb")
    nc.scalar.dma_start(resb_t[:], dt["res_b"])
    fh_t = cst.tile([128, 2 * RC], BF16, tag="fh")
    nc.scalar.dma_start(fh_t[:], dt["fh"])

    # per-level DCN weights, loaded once
    dcn_ts = []
    for lvl in range(2):
        t_ = cst.tile([128, NT * 2 * 2 * 128], BF16, tag=f"dcn{lvl}")
        nc.scalar.dma_start(t_[:], dt["dcn_w"][lvl])
        dcn_ts.append(t_[:].rearrange("p (k i o q) -> p k i o q",
                                      k=NT, i=2, o=2, q=128))

    fp_ap = {0: dt["fp1"], 1: dt["fp0"]}

    # DRAM staging tile for wrapped gather indices; zero it once so the
    # per-(t,z) pad lanes (cc 30:32) read as index 0 in every call.
    repD = drp.tile([16, NT * 64], I16, tag="repD")
    zs16 = smi.tile([16, NT * 64], I16, tag="zs16")
    nc.vector.memset(zs16[:], 0)
    nc.sync.dma_start(repD[:], zs16[:])

    # ---- per-call ---------------------------------------------------------
    for ci, lvl in enumerate(CALLS):
        Win = HIN[lvl]
        Wt = TW[lvl]
        dcn_v = dcn_ts[lvl]

        # offset conv: om_ps rows 0:48, (z,512)-chunked, 480 used
        om_ps = pso.tile([128, 1024], F32, tag="ps", name=f"omps_{ci}")
        conv3x3(nc, fsh, lambda ti, ih: com_v[:, lvl, ti, ih], om_ps, rows=48)

        # mask activation (com_b offset-bias is folded into byx host-side)
        m16 = omp.tile([16, RC], BF16, tag="m16")
        omv1 = om_ps[32:48, :].rearrange("p (z c) -> p z c", z=2)[:, :, 0:480]
        nc.scalar.activation(m16[:], omv1, AF.Sigmoid,
                             bias=comb_t[32:48, lvl:lvl + 1])

        # stage offsets PSUM->SBUF bf16 (on DVE, parallel with the sigmoid),
        # then shuffle into [64,480] / [32,480] via bf16 PE perm matmuls
        om01 = omp.tile([32, RC], BF16, tag="om01")
        omv0 = om_ps[0:32, :].rearrange("p (z c) -> p z c", z=2)[:, :, 0:480]
        nc.vector.tensor_copy(om01[:], omv0)
        pos0ps = pso.tile([128, 1024], F32, tag="ps", name=f"pos0ps_{ci}")
        for yx in range(2):
            for rcb in range(2):
                nc.tensor.matmul(
                    pos0ps[yx * 32:(yx + 1) * 32, 0:480],
                    pperm_v[:, yx * 2 + rcb, :],
                    om01[0:32, rcb * 480:(rcb + 1) * 480],
                    start=(rcb == 0), stop=(rcb == 1))
        pos0 = pos0ps[0:64, 0:480]
        m32ps = pso.tile([128, 1024], F32, tag="ps", name=f"m32ps_{ci}")
        for rcb in range(2):
            nc.tensor.matmul(m32ps[0:32, 0:480], mperm_v[:, rcb, :],
                             m16[:, rcb * 480:(rcb + 1) * 480],
                             start=(rcb == 0), stop=(rcb == 1))
        m32 = m32ps[0:32, 0:480]

        # ---- small math ----
        cnt = [0]

        def t64():
            cnt[0] += 1
            return s64p.tile([64, 480], F32, tag="s64", name=f"t64_{ci}_{cnt[0]}")

        def t32():
            cnt[0] += 1
            return s32p.tile([32, 480], F32, tag="s32", name=f"t32_{ci}_{cnt[0]}")

        def t64i():
            cnt[0] += 1
            return s64p.tile([64, 480], I32, tag="s64i", bufs=1,
                             name=f"t64i_{ci}_{cnt[0]}")

        # positions carry a +1+1024 shift (baked into byx): +1 for the grid,
        # +1024 so floor-via-mod sees positive operands on hardware.
        # --- idx-critical path first (high priority: gathers wait on it) ---
        hp = tc.high_priority()
        hp.__enter__()
        sh = t64()
        nc.vector.tensor_tensor(sh[:], pos0,
                                byx_t[:, lvl * 480:(lvl + 1) * 480], A.add)
        i32t = t64i()
        nc.vector.tensor_copy(i32t[:], sh[:])
        ff = t64()
        nc.vector.tensor_copy(ff[:], i32t[:])
        gt = t64()
        nc.vector.tensor_tensor(gt[:], ff[:], sh[:], A.is_gt)
        fls = t64()
        nc.vector.tensor_tensor(fls[:], ff[:], gt[:], A.subtract)
        c0 = t64()
        nc.vector.tensor_scalar(c0[:], fls[:], 1024.0, hi0_t[:, lvl:lvl + 1],
                                A.max, A.min)
        # gather idx = (c0y-1024)*Wt + c0x-1024  (psx pre-subtracts the shift)
        psx = t32()
        nc.vector.tensor_scalar(psx[:], c0[32:64, :],
                                -1024.0 * (Wt + 1.0), None, A.add)
        gyt = t32()
        nc.vector.scalar_tensor_tensor(gyt[:], c0[0:32, :], float(Wt),
                                       psx[:], A.mult, A.add)
        i16t = smi.tile([32, 480], I16, tag="i16")
        nc.vector.tensor_copy(i16t[:], gyt[:])
        dflat = drp.tile([32, 480], I16, tag="dfl")
        nc.sync.dma_start(dflat[:], i16t[:])

        # idx wrap via DRAM: repD[p', t*64+z*32+cc] = dflat[(z*16+t)*480
        # + cc*16 + p'] (DRAM->DRAM strided, chunked by tap-half x z on two
        # queues), then broadcast DMAs fill the 8 replica row-groups.
        # repD pad lanes (cc 30:32) are zeroed once at kernel start.
        dfv = dflat[:].rearrange("p c -> (p c)")
        rdv = repD[:].rearrange("p (t z cc) -> p t z cc", t=NT, z=2, cc=32)
        HT = NT // 2
        for th, eng in ((0, nc.sync), (1, nc.scalar)):
            for z in range(2):
                wrap = smi.tile([16, HT * 30], I16, tag=f"wrap{th}{z}",
                                name=f"wrap_{ci}_{th}_{z}")
                base = (z * 16 + th * HT) * 480
                src = dfv[base:base + HT * 480]
                src = src.rearrange("(tc p) -> p tc", p=16)
                eng.dma_start(wrap[:], src)
                wv_ = wrap[:].rearrange("p (t cc) -> p t cc", t=HT)
                eng.dma_start(rdv[0:16, th * HT:(th + 1) * HT, z, 0:30], wv_)
        rep = smi.tile([128, NT * 64], I16, tag="rep")
        for grp in range(8):
            eng = (nc.sync, nc.scalar, nc.gpsimd, nc.sync)[grp % 4]
            eng.dma_start(rep[grp * 16:(grp + 1) * 16, :], repD[:])
        hp.__exit__(None, None, None)

        # --- weight path (overlaps the idx DMA chain) ---
        frac = t64()
        nc.vector.tensor_tensor(frac[:], sh[:], fls[:], A.subtract)
        V = t64()
        nc.vector.tensor_tensor(V[:], c0[:], fls[:], A.is_equal)
        u = t64()
        nc.vector.tensor_scalar(u[:], frac[:], -1.0, 1.0, A.mult, A.add)

        # mask' = m * Vy * Vx  (x rows copied down to base partition 0;
        # weight-path copies on Act, off the DVE critical path)
        vx32 = t32()
        nc.scalar.copy(vx32[:], V[32:64, :])
        mv = t32()
        nc.vector.tensor_tensor(mv[:], m32, V[0:32, :], A.mult)
        mm_ = t32()
        nc.vector.tensor_tensor(mm_[:], mv[:], vx32[:], A.mult)
        A0 = t32()
        nc.vector.tensor_tensor(A0[:], u[0:32, :], mm_[:], A.mult)
        A1 = t32()
        nc.vector.tensor_tensor(A1[:], frac[0:32, :], mm_[:], A.mult)
        xs0 = t32()
        nc.scalar.copy(xs0[:], u[32:64, :])
        xs1 = t32()
        nc.scalar.copy(xs1[:], frac[32:64, :])

        # wall [32, (cy, px, 480)] bf16
        wall = walp.tile([32, 4 * 480], BF16, tag="wall")
        nc.vector.tensor_tensor(wall[:, 0 * 480:1 * 480], A0[:], xs0[:], A.mult)
        nc.vector.tensor_tensor(wall[:, 1 * 480:2 * 480], A0[:], xs1[:], A.mult)
        nc.vector.tensor_tensor(wall[:, 2 * 480:3 * 480], A1[:], xs0[:], A.mult)
        nc.vector.tensor_tensor(wall[:, 3 * 480:4 * 480], A1[:], xs1[:], A.mult)

        # dc accumulator [2][128, 1024] ((z,512)-chunked, 480 used)
        dcs = [psd.tile([128, 1024], F32, tag=f"dc{oh}", name=f"dc_{ci}_{oh}")
               for oh in range(2)]

        fpv = fp_ap[lvl]

        def emit_bcast(t):
            # PE broadcast via one-hot selector: bc[o,c] = wall[t+16z, c]
            wallb = wbp.tile([128, 4 * 960], BF16, tag="wallb",
                             name=f"wallb_{ci}_{t}")
            for j in range(4):
                bc = pso.tile([128, 1024], F32, tag="ps", name=f"bc_{ci}_{t}_{j}")
                for z in range(2):
                    nc.tensor.matmul(bc[:, z * 512:z * 512 + 480],
                                     sel_v[:, t + 16 * z, :],
                                     wall[0:32, j * 480:(j + 1) * 480],
                                     start=True, stop=True)
                bcv = bc[:].rearrange("p (z c) -> p z c", z=2)[:, :, 0:480]
                wbv = wallb[:, j * 960:(j + 1) * 960].rearrange(
                    "p (z c) -> p z c", z=2)
                nc.scalar.activation(wbv, bcv, AF.Copy)
            return wallb

        def emit_gather(t, z):
            # gather: one 2KB element per sample = full 2x2 patch; half-tap
            g = gat.tile([128, 8 * 512], BF16, tag="g", name=f"g_{ci}_{t}_{z}")
            gv = g[:].rearrange("p (j i) -> p j i", j=8)
            nc.gpsimd.dma_gather(gv, fpv,
                                 rep[:, t * 64 + z * 32:t * 64 + z * 32 + 32],
                                 512, 512, 1024, transpose=True,
                                 single_packet=False)
            return g

        # all gather dispatches up-front: Pool's in-order queue paces them
        # purely by gat-buffer WAR, never behind a compute op
        gs = {(t, z): emit_gather(t, z) for t in range(NT) for z in range(2)}
        wallbs = {0: emit_bcast(0)}
        for t in range(NT):
            if t + 1 < NT:
                wallbs[t + 1] = emit_bcast(t + 1)
            wallb = wallbs.pop(t)
            for z in range(2):
                g = gs.pop((t, z))
                gb = g[:]

                # in-place mul: p = g * wall  (one op, both corners)
                pv = vp(gb, [[4096, 128], [2048, 2], [1024, 2], [512, 2],
                             [1, 480]])
                wv = vp(wallb[:], [[3840, 128], [1920, 2], [960, 2], [0, 2],
                                   [1, 480]], doff=z * 480)
                nc.vector.tensor_tensor(pv, pv, wv, A.mult)

                # q = p[cy0] + p[cy1]   [128, (px, hl, 480)]
                # z0 on DVE, z1 on Pool
                q = qp.tile([128, 2 * 960], BF16, tag="q",
                            name=f"q_{ci}_{t}_{z}")
                qv = vp(q[:], [[1920, 128], [960, 2], [480, 2], [1, 480]])
                pa = vp(gb, [[4096, 128], [1024, 2], [512, 2], [1, 480]])
                pb = vp(gb, [[4096, 128], [1024, 2], [512, 2], [1, 480]],
                        doff=2048)
                nc.vector.tensor_tensor(qv, pa, pb, A.add)

                # s-sum folded into the matmuls: feed both px halves of q
                qview = q[:].rearrange("p (x h c) -> p x h c", x=2, h=2)
                for oh in range(2):
                    for ih in range(2):
                        for px in range(2):
                            nc.tensor.matmul(
                                dcs[oh][:, z * 512:z * 512 + 480],
                                dcn_v[:, t, ih, oh],
                                qview[:, px, ih, :],
                                start=(t == 0 and ih == 0 and px == 0),
                                stop=(t == NT - 1 and ih == 1 and px == 1))

        # f update: f += relu(dc + b)   (bf16 master; h1 add on Pool so the
        # two halves update in parallel and the conv starts sooner)
        for h in range(2):
            rel = fup.tile([128, RC], BF16, tag="rel", name=f"rel_{ci}_{h}")
            dcv = dcs[h][:].rearrange("p (z c) -> p z c", z=2)[:, :, 0:480]
            nc.scalar.activation(rel[:], dcv, AF.Relu,
                                 bias=dcnb_t[:, 2 * lvl + h:2 * lvl + h + 1])
            fsv = fsh[h][:].rearrange("p (r c) -> p r c", c=FW)[:, 1:25, 1:41]
            rv = rel[:].rearrange("p (r c) -> p r c", c=HOUT)
            (nc.vector if h == 0 else nc.gpsimd).tensor_tensor(
                fsv, fsv, rv, A.add)

    # ---- residual conv + fh ----------------------------------------------
    # fh is pre-accumulated into the PSUM via an identity matmul (start),
    # then the conv taps accumulate on top; output = act(psum + bias).
    res_t = wgt.tile([128, 9 * 2 * 2 * 128], BF16, tag="res")
    nc.sync.dma_start(res_t[:], dt["res_w"])
    res_v = res_t[:].rearrange("p (t i o q) -> p t i o q", t=9, i=2, o=2)
    for oh in range(2):
        rps = psd.tile([128, 1024], F32, tag=f"dc{oh}", name=f"rps_{oh}")
        fhv = fh_t[:].rearrange("p (o z c) -> p o z c", o=2, z=2)
        for z in range(2):
            nc.tensor.matmul(rps[:, z * 512:z * 512 + 480], ident_t[:],
                             fhv[:, oh, z, :], start=True, stop=False)
        conv3x3(nc, fsh, lambda ti, ih, oh=oh: res_v[:, ti, ih, oh], rps,
                accum=True)
        ot = fup.tile([128, RC], F32, tag="ot")
        rpv = rps[:].rearrange("p (z c) -> p z c", z=2)[:, :, 0:480]
        nc.scalar.activation(ot[:], rpv, AF.Identity, bias=resb_t[:, oh:oh + 1])
        nc.sync.dma_start(out_d[128 * oh:128 * (oh + 1), :], ot[:])


def conv3x3(nc, fsh, w_fn, out_ps, rows=128, accum=False):
    """3x3 stride-1 conv over the padded f window; out [rows, (z,512|480)].

    ih-outer so the ih=0 matmuls can start before fsh[1] is updated."""
    taps = [(a, b) for a in (-1, 0, 1) for b in (-1, 0, 1)]
    for ih in range(2):
        rhs = fsh[ih][:].rearrange("p (r c) -> p r c", c=FW)
        for ti, (dy, dx) in enumerate(taps):
            for nh in range(2):
                nc.tensor.matmul(
                    out_ps[0:rows, nh * 512:nh * 512 + 480],
                    w_fn(ti, ih),
                    rhs[:, 1 + dy + nh * 12:1 + dy + nh * 12 + 12,
                        1 + dx:1 + dx + 40],
                    start=(not accum and ih == 0 and ti == 0),
                    stop=(ih == 1 and ti == 8))


# ===========================================================================
# host side
# ===========================================================================

def packed_table(f):
    """[(H+1)*(W+1), 1024] bf16: entry (yy,xx) = 2x2 patch at (yy-1, xx-1)."""
    Cc, H, W = f.shape
    fpad = np.zeros((Cc, H + 2, W + 2), np.float32)
    fpad[:, 1:H + 1, 1:W + 1] = f
    parts = [fpad[:, dy:dy + H + 1, dx:dx + W + 1]
             for dy, dx in ((0, 0), (0, 1), (1, 0), (1, 1))]
    t = np.stack(parts, axis=0)            # [4, C, H+1, W+1]
    t = t.transpose(2, 3, 0, 1)            # [H+1, W+1, 4, C]
    return np.ascontiguousarray(
        t.reshape((H + 1) * (W + 1), 4 * Cc)).astype(ml_dtypes.bfloat16)


def prep_core_inputs(inputs, b, half):
    """Per-core input map for image b, row-half `half` (0=top)."""
    g0 = 0 if half == 0 else 16
    f0 = np.asarray(inputs["f0"][b], np.float32)
    f1 = np.asarray(inputs["f1"][b], np.float32)
    f2 = np.asarray(inputs["f2"][b], np.float32)

    finit = np.zeros((C, FR, FW), np.float32)
    for r in range(FR):
        gr = g0 - 1 + r
        if 0 <= gr < HOUT:
            finit[:, r, 1:41] = f2[:, gr, :]

    # fh as [128, (oh, rc)]
    fh0 = f2[:, g0:g0 + ROWS, :].reshape(C, RC)
    fh = np.concatenate([fh0[:128], fh0[128:]], axis=1)

    perm = list(range(0, 32, 2)) + list(range(1, 32, 2)) + list(range(32, 48))

    # base positions in +1-shifted grid coords; offset-conv bias folded in
    byx = np.zeros((2, 64, 480), np.float32)
    hi0 = np.zeros((2, 64, 1), np.float32)
    for lvl in range(2):
        k_, st_, pad_, dil_ = CONFIGS[lvl]
        Hin = HIN[lvl]
        cbp = np.asarray(inputs[f"com_b{lvl}"], np.float32)[perm]
        rc = np.arange(480)
        for rcb in range(2):
            rr = (rcb * 480 + rc) // HOUT
            cc = (rcb * 480 + rc) % HOUT
            for t in range(NT):
                byx[lvl, rcb * 16 + t] = (st_ * (g0 + rr) - pad_
                                          + (t // k_) * dil_ + 1025 + cbp[t])
                byx[lvl, 32 + rcb * 16 + t] = (st_ * cc - pad_ + (t % k_) * dil_
                                               + 1025 + cbp[16 + t])
        hi0[lvl, 0:32] = 1024 + Hin   # clamp hi in shifted coords
        hi0[lvl, 32:64] = 1024 + Hin
    byx = byx.transpose(1, 0, 2).reshape(64, 2 * 480)
    hi0 = hi0.transpose(1, 0, 2).reshape(64, 2)
    com_w = np.zeros((2, 9, 2, 128, 48), np.float32)
    com_b = np.zeros((2, 48, 1), np.float32)
    dcn_w = np.zeros((2, NT, 2, 2, 128, 128), np.float32)
    dcn_b = np.zeros((2, 2, 128, 1), np.float32)
    for lvl in range(2):
        cw = np.asarray(inputs[f"com_w{lvl}"], np.float32)[perm]
        cb = np.asarray(inputs[f"com_b{lvl}"], np.float32)[perm]
        for ty in range(3):
            for tx in range(3):
                for ih in range(2):
                    com_w[lvl, ty * 3 + tx, ih] = \
                        cw[:, ih * 128:(ih + 1) * 128, ty, tx].T
        com_b[lvl, :, 0] = cb
        dw = np.asarray(inputs[f"dcn_w{lvl}"], np.float32)
        for k in range(NT):
            for ih in range(2):
                for oh in range(2):
                    dcn_w[lvl, k, ih, oh] = dw[oh * 128:(oh + 1) * 128,
                                               ih * 128:(ih + 1) * 128,
                                               k // 4, k % 4].T
        db = np.asarray(inputs[f"dcn_b{lvl}"], np.float32)
        dcn_b[lvl, 0, :, 0] = db[:128]
        dcn_b[lvl, 1, :, 0] = db[128:]
    rw = np.asarray(inputs["res_w"], np.float32)
    res_w = np.zeros((9, 2, 2, 128, 128), np.float32)
    for ty in range(3):
        for tx in range(3):
            for ih in range(2):
                for oh in range(2):
                    res_w[ty * 3 + tx, ih, oh] = rw[oh * 128:(oh + 1) * 128,
                                                    ih * 128:(ih + 1) * 128,
                                                    ty, tx].T
    rb = np.asarray(inputs["res_b"], np.float32)
    res_b = np.stack([rb[:128], rb[128:]], axis=1)  # [128, 2]

    # PE permutation matrices for the om -> pos0 / m16 -> m32 shuffles
    pperm = np.zeros((32, 4, 32), ml_dtypes.bfloat16)
    for yx in range(2):
        for rcb in range(2):
            for t in range(16):
                pperm[yx * 16 + t, yx * 2 + rcb, rcb * 16 + t] = 1.0
    mperm = np.zeros((16, 2, 32), ml_dtypes.bfloat16)
    for rcb in range(2):
        for t in range(16):
            mperm[t, rcb, rcb * 16 + t] = 1.0

    com_w = com_w.transpose(3, 0, 1, 2, 4).reshape(128, -1)
    com_b = com_b.transpose(1, 0, 2).reshape(48, 2)
    dcn_w = dcn_w.transpose(0, 4, 1, 2, 3, 5).reshape(2, 128, -1)
    dcn_b = dcn_b.transpose(2, 0, 1, 3).reshape(128, 4)
    res_w = res_w.transpose(3, 0, 1, 2, 4).reshape(128, -1)

    return {
        "fp0": packed_table(f0),
        "fp1": packed_table(f1),
        "finit": finit.reshape(C, FSZ).astype(ml_dtypes.bfloat16),
        "fh": fh.astype(ml_dtypes.bfloat16),
        "byx": byx,
        "hi0": hi0,
        "sel": np.ascontiguousarray(
            np.tile(np.eye(32, dtype=np.float32)[:, :, None],
                    (1, 1, 128)).reshape(32, 32 * 128)
        ).astype(ml_dtypes.bfloat16),
        "pperm": pperm.reshape(32, 4 * 32),
        "mperm": mperm.reshape(16, 2 * 32),
        "ident": np.eye(128, dtype=np.float32).astype(ml_dtypes.bfloat16),
        "com_w": com_w.astype(ml_dtypes.bfloat16),
        "com_b": np.ascontiguousarray(com_b),
        "dcn_w": np.ascontiguousarray(dcn_w).astype(ml_dtypes.bfloat16),
        "dcn_b": np.ascontiguousarray(dcn_b),
        "res_w": np.ascontiguousarray(res_w).astype(ml_dtypes.bfloat16),
        "res_b": np.ascontiguousarray(res_b).astype(np.float32),
    }


def assemble_output(results):
    out = np.zeros((B, C, HOUT, HOUT), np.float32)
    for b in range(B):
        top = np.asarray(results[2 * b]["out"]).reshape(C, ROWS, HOUT)
        bot = np.asarray(results[2 * b + 1]["out"]).reshape(C, ROWS, HOUT)
        out[b, :, 0:20, :] = top[:, 0:20, :]
        out[b, :, 20:40, :] = bot[:, 4:24, :]
    return out


_NC_CACHE = []


def kernel(**inputs):
    if not _NC_CACHE:
        _NC_CACHE.append(build_program())
    nc = _NC_CACHE[0]
    in_maps = [prep_core_inputs(inputs, b, half)
               for b in range(B) for half in range(2)]
    from concourse.bass_utils import run_bass_kernel_spmd
    r = run_bass_kernel_spmd(nc, in_maps, list(range(8)))
    return assemble_output(r.results)
